# revision 87
# baseline (speedup 1.0000x reference)
"""Trainium2 Bass kernel: batched multi-head cross-attention.

Reference computation (per batch element b of 8, one NeuronCore each):
    K,V from x; Q from y (heads=16, dim=1024, d_head=64, scale=dim**-0.5)
    out = softmax(Q K^T * scale) V  -> concat heads -> @ w_out.T + b_out

Sharding: pure data-parallel on batch (8 batch elements -> 8 cores).
No collectives. All layout transposes are done host-side:

  xT, yT   : [dim, n]        (feature-major inputs)
  wqkP     : [dim, 16*128]   w_qkv.T Q|K columns regrouped per head pair:
                             block 2t = Q cols of pair t (pre-scaled),
                             block 2t+1 = K cols of pair t.  Pair-t blocks
                             are DMA'd t-ordered so pair 0/1 projections
                             (which gate the whole exp stream) start first.
  wvT      : [dim, dim]
  woutT    : [dim, dim]      = w_out.T   (loaded late, reuses wv space)
  biasb    : [128, dim]      = b_out broadcast over partitions

Device pipeline per core (bf16 matmuls, fp32 PSUM accumulation):
  Q^T[f,i] = wqT.T @ yT            (scale folded into wqT on host)
  K^T[f,j] = wkT.T @ xT
  V[j,f]   = xT.T @ wvT            (token-major, + ones column per head)
  dots^T[j,i] = k_h^T.T @ q_h^T    (K=64) -- the even/odd heads of a pair
      write the two halves of ONE [128,1024] psum tile, so both matmuls
      share a single dependency gate, issue back-to-back, and execute
      CONCURRENTLY on disjoint PE row strips (0,0)/(64,0): 2x dots rate.
  exp on ScalarE psum->sbuf bf16   ([128,1024] activations; ScalarE is the
      co-critical engine at ~144us total)
  [out_h^T; s_h] = [v_h|1].T @ exp   (M=65: row 64 = softmax denominator)
  out_h^T *= 1/s_h   (reciprocal_approx_fast + gpsimd partition_broadcast)
  res[i,g] = O^T.T @ woutT + bias    (natural layout, contiguous DMA out)

Scheduling: engine queues are in-order, so PE emission order ~= PE
execution order.  The dots+exp stream is ScalarE-paced, so between dots
windows we pump "filler" PE units (attn@V of the previous pair first --
that recycles ex tiles -- then V projection / Q,K projection two pairs
ahead) from a deque.
"""

from collections import deque
from contextlib import ExitStack

import numpy as np
import ml_dtypes

import concourse.bass as bass
import concourse.tile as tile
from concourse import bacc, mybir
from concourse.bass_utils import run_bass_kernel_spmd

DIM = 1024
N = 1024
HEADS = 16
DH = 64  # head dim
SCALE = DIM ** -0.5
P = 128          # partitions
NH = 512         # free-dim half (one PSUM bank of fp32)
BF16 = mybir.dt.bfloat16
F32 = mybir.dt.float32
EXP = mybir.ActivationFunctionType.Exp


def build_attention_nc():
    nc = bacc.Bacc("TRN2", target_bir_lowering=False, debug=False)

    xT_d = nc.dram_tensor("xT", [DIM, N], BF16, kind="ExternalInput")
    yT_d = nc.dram_tensor("yT", [DIM, N], BF16, kind="ExternalInput")
    # wqkR[t, p, c, col]: pair-t block, partition p, contraction tile c.
    # One full-bandwidth [128, 2048] DMA per head pair.
    wqkP_d = nc.dram_tensor("wqkP", [FT_HOST, P, 2 * DIM], BF16,
                            kind="ExternalInput")
    wvT_d = nc.dram_tensor("wvT", [DIM, DIM], BF16, kind="ExternalInput")
    woutT_d = nc.dram_tensor("woutT", [DIM, DIM], BF16, kind="ExternalInput")
    biasb_d = nc.dram_tensor("biasb", [P, DIM], F32, kind="ExternalInput")
    out_d = nc.dram_tensor("out", [N, DIM], F32, kind="ExternalOutput")

    CT = DIM // P   # 8 contraction tiles
    FT = DIM // P   # 8 head pairs
    JT = N // P     # 8 key-token tiles
    IT = N // P     # 8 query-token tiles

    with TileBuild(nc) as b:
        tc = b.tc
        ctx = b.ctx

        wqk_pool = ctx.enter_context(tc.tile_pool(name="wqk", bufs=FT))
        xy_pool = ctx.enter_context(tc.tile_pool(name="xy", bufs=2 * CT))
        qk_pool = ctx.enter_context(tc.tile_pool(name="qk", bufs=6))
        va_pool = ctx.enter_context(tc.tile_pool(name="va", bufs=JT))
        ex_pool = ctx.enter_context(tc.tile_pool(name="ex", bufs=20))
        ot_pool = ctx.enter_context(tc.tile_pool(name="ot", bufs=FT))
        sm_pool = ctx.enter_context(tc.tile_pool(name="sm", bufs=2))
        res_pool = ctx.enter_context(tc.tile_pool(name="res", bufs=2))
        # PSUM: 8 banks of [128, 2KB].  pd: dots pair tiles ([128,1024] f32
        # = 2 banks, 2 in flight), pp: projections/outproj ([128,512], 2),
        # pa: attn@V accumulators ([128,512], 2).
        pd_pool = ctx.enter_context(tc.tile_pool(name="pd", bufs=2, space="PSUM"))
        pp_pool = ctx.enter_context(tc.tile_pool(name="pp", bufs=2, space="PSUM"))
        pa_pool = ctx.enter_context(tc.tile_pool(name="pa", bufs=2, space="PSUM"))

        # ---- input DMA, two queues ----
        # sync queue:   wqk pair 0, pair 1, y, wqk pairs 2-7, bias
        # scalar queue: x, wv, (later) wout
        # DMA-completion waits are per-queue prefix-coarse, so the K
        # projections of pairs 0/1 unblock after ~1MB+x, Q after +y.
        wv_pool = ctx.enter_context(tc.tile_pool(name="wv", bufs=CT))
        wo_pool = ctx.enter_context(tc.tile_pool(name="wo", bufs=FT))
        wv_t, xT_t, yT_t = [], [], []
        wqk_t = []  # [t] -> [128, 2048] (c-major blocks of 256: [Q|K] cols)

        def dma_wqk(t):
            w = wqk_pool.tile([P, 2 * DIM], BF16, tag="wqk", name=f"wqk{t}")
            nc.sync.dma_start(w[:], wqkP_d[t, :, :])
            wqk_t.append(w)

        dma_wqk(0)
        dma_wqk(1)
        for c in range(CT):
            t = xy_pool.tile([P, N], BF16, tag="xy", name=f"xt{c}")
            nc.scalar.dma_start(t[:], xT_d[c * P:(c + 1) * P, :])
            xT_t.append(t)
        for c in range(CT):
            t = xy_pool.tile([P, N], BF16, tag="xy", name=f"yt{c}")
            nc.scalar.dma_start(t[:], yT_d[c * P:(c + 1) * P, :])
            yT_t.append(t)
        for c in range(CT):
            t = wv_pool.tile([P, DIM], BF16, tag="wv", name=f"wv{c}")
            nc.scalar.dma_start(t[:], wvT_d[c * P:(c + 1) * P, :])
            wv_t.append(t)
        for t in range(2, FT):
            dma_wqk(t)
        wo_t = []
        for f in range(FT):
            w = wo_pool.tile([P, DIM], BF16, tag="wo", name=f"wo{f}")
            nc.sync.dma_start(w[:], woutT_d[f * P:(f + 1) * P, :])
            wo_t.append(w)
        bias_t = res_pool.tile([P, DIM], F32, tag="bias", bufs=1)
        nc.sync.dma_start(bias_t[:], biasb_d[:, :])

        # ---- filler units: callables emitting ~8 PE matmuls each ----
        # Two priorities: attn@V units (hi) unblock the ex-tile ring, so
        # they preempt the projection backlog (lo) -- but only after aging
        # two dots windows, so the exp tiles they read have drained and
        # their matmuls don't stall the in-order PE queue.
        filler = deque()      # lo: projections / V projection
        filler_hi = deque()   # hi: (enqueue_window, attn@V unit)
        win = [0]

        def pump(k):
            for _ in range(k):
                if filler_hi and win[0] - filler_hi[0][0] >= 2:
                    filler_hi.popleft()[1]()
                elif filler:
                    filler.popleft()()
                elif filler_hi:
                    filler_hi.popleft()[1]()

        # Q/K projection for head pair t, one (which, half) per unit.
        QK = {}  # t -> [QTt, KTt]
        proj_pending = {}  # t -> number of un-run units

        def enqueue_proj(t):
            proj_pending[t] = 4
            QK[t] = [
                qk_pool.tile([P, N], BF16, tag="qk", name=f"qk{w}_{t}")
                for w in range(2)
            ]

            qk_tiles = QK[t]

            def unit(which, n, t=t):
                rhs_t = yT_t if which == 0 else xT_t
                ps = pp_pool.tile([P, NH], F32, tag="pp", name=f"psp{which}{n}_{t}")
                for c in range(CT):
                    nc.tensor.matmul(
                        ps[:],
                        lhsT=wqk_t[t][:, c * 256 + which * P:
                                       c * 256 + (which + 1) * P],
                        rhs=rhs_t[c][:, n * NH:(n + 1) * NH],
                        start=(c == 0), stop=(c == CT - 1),
                    )
                nc.vector.tensor_copy(
                    qk_tiles[which][:, n * NH:(n + 1) * NH], ps[:])
                proj_pending[t] -= 1

            # K (from x) first: the x DMA queue finishes before y's.
            for which in (1, 0):
                for n in range(2):
                    filler.append(lambda w=which, n=n: unit(w, n))

        # V projection for key tile vj: token-major + per-head ones column.
        VA = [None] * JT
        vproj_pending = [0]

        def enqueue_vproj(vj):
            vproj_pending[0] += 2
            va = va_pool.tile([P, HEADS, DH + 1], BF16, tag="va", name=f"va{vj}")
            VA[vj] = va

            def unit(n, vj=vj, va=va):
                ps = pp_pool.tile([P, NH], F32, tag="pp", name=f"psv{vj}_{n}")
                for c in range(CT):
                    nc.tensor.matmul(
                        ps[:],
                        lhsT=xT_t[c][:, vj * P:(vj + 1) * P],
                        rhs=wv_t[c][:, n * NH:(n + 1) * NH],
                        start=(c == 0), stop=(c == CT - 1),
                    )
                # half n covers heads 8n..8n+7 (512 = 8 heads x 64)
                nc.vector.tensor_copy(
                    va[:, 8 * n:8 * (n + 1), 0:DH],
                    ps[:].rearrange("p (h c) -> p h c", c=DH),
                )
                if n == 1:
                    nc.vector.memset(va[:, :, DH:DH + 1], 1.0)
                vproj_pending[0] -= 1

            for n in range(2):
                filler.append(lambda n=n: unit(n))

        # attn @ V + softmax-normalize for one head pair.  ex_t[n][j] is the
        # shared exp tile of (key tile j, half n); head par reads columns
        # [par*512:(par+1)*512].  The ones column of VA makes psum row 64
        # the softmax denominator.
        OT = [ot_pool.tile([P, N], BF16, tag="ot", name=f"OT{f}") for f in range(FT)]

        def enqueue_attnv(t, ex_t):
            def unit(par, n, t=t):
                h = 2 * t + par
                acc = pa_pool.tile([P, NH], F32, tag="pa", name=f"acc{h}_{n}")
                for j in range(JT):
                    nc.tensor.matmul(
                        acc[0:DH + 1, :],
                        lhsT=VA[j][:, h, :],
                        rhs=ex_t[n][j][:, par * NH:(par + 1) * NH],
                        start=(j == 0), stop=(j == JT - 1),
                    )
                # r = 1/s; HW custom ops only honor partition base 0, so
                # hop s: psum[64] -> sbuf[64] -> DMA -> sbuf[0].
                s_sb = sm_pool.tile([DH + 1, NH], F32, tag="ss", name=f"ss{h}_{n}")
                nc.vector.tensor_copy(s_sb[DH:DH + 1, :], acc[DH:DH + 1, :])
                r_sb = sm_pool.tile([1, NH], F32, tag="rs", name=f"rs{h}_{n}")
                nc.sync.dma_start(out=r_sb[0:1, :], in_=s_sb[DH:DH + 1, :])
                # recip result parked in s_sb's unused partition-0 row
                nc.vector.reciprocal_approx_fast(
                    out=s_sb[0:1, 0:NH], in_=r_sb[0:1, :])
                rb = sm_pool.tile([DH, NH], F32, tag="rb", name=f"rb{h}_{n}")
                nc.gpsimd.partition_broadcast(rb[:], s_sb[0:1, 0:NH])
                if par == 0:
                    nc.vector.tensor_mul(
                        OT[t][0:DH, n * NH:(n + 1) * NH], acc[0:DH, :], rb[:])
                else:
                    om = sm_pool.tile([DH, NH], BF16, tag="om", name=f"om{h}_{n}")
                    nc.vector.tensor_mul(om[:], acc[0:DH, :], rb[:])
                    nc.sync.dma_start(
                        out=OT[t][DH:P, n * NH:(n + 1) * NH], in_=om[:])

            return unit

        # ---- dots + exp for head pair t (the ScalarE-paced main stream).
        # One [128,1024] psum tile per (j, n): even head in cols 0:512, odd
        # head in cols 512:1024.  Both matmuls share the tile's dependency
        # gate -> issued back-to-back -> concurrent on row strips 0-1/2-3.
        def emit_dots(t):
            while proj_pending[t] > 0:
                pump(1)
            QTt, KTt = QK.pop(t)
            ex_t = ([], [])  # [n][j]
            attnv_unit = enqueue_attnv(t, ex_t)
            for n in range(2):
                for j in range(JT):
                    ps = pd_pool.tile([P, N], F32, tag="pd",
                                      name=f"psd{t}_{j}_{n}")
                    for par, pb in ((0, 0), (1, DH)):
                        nc.tensor.matmul(
                            ps[:, par * NH:(par + 1) * NH],
                            lhsT=KTt[pb:pb + DH, j * P:(j + 1) * P],
                            rhs=QTt[pb:pb + DH, n * NH:(n + 1) * NH],
                            start=True, stop=True,
                        )
                    ex = ex_pool.tile([P, N], BF16, tag="ex",
                                      name=f"ex{t}_{j}_{n}")
                    nc.scalar.activation(ex[:], ps[:], EXP)
                    ex_t[n].append(ex)
                    # this pair's half-n attn@V is enqueued once its 8 exp
                    # tiles are in flight (behind any still-pending V
                    # projections, which write the VA tiles it reads)
                    if j == JT - 1:
                        if vproj_pending[0] > 0:
                            filler.append(lambda nn=n: attnv_unit(0, nn))
                            filler.append(lambda nn=n: attnv_unit(1, nn))
                        else:
                            filler_hi.append((win[0], lambda nn=n: attnv_unit(0, nn)))
                            filler_hi.append((win[0], lambda nn=n: attnv_unit(1, nn)))
                    # one filler unit per window, but keep the first window
                    # of each pair clear so its dots/exp restart promptly
                    win[0] += 1
                    if 8 * n + j >= 1:
                        pump(1)
            return ex_t

        # ---- pipeline ----
        enqueue_proj(0)
        enqueue_proj(1)
        pump(4)  # pair-0 projection gates the stream; pair 1 fills early
        for t in range(FT):
            if t + 2 < FT:
                enqueue_proj(t + 2)
            if t == 0:
                for vj in range(JT):
                    enqueue_vproj(vj)
            emit_dots(t)
        pump(len(filler) + len(filler_hi))

        # ---- output projection + bias ----
        for i in range(IT):
            res = res_pool.tile([P, DIM], F32, tag="res", name=f"res{i}")
            for n in range(2):
                ps = pp_pool.tile([P, NH], F32, tag="pp", name=f"psf{i}_{n}")
                for f in range(FT):
                    nc.tensor.matmul(
                        ps[:],
                        lhsT=OT[f][:, i * P:(i + 1) * P],
                        rhs=wo_t[f][:, n * NH:(n + 1) * NH],
                        start=(f == 0), stop=(f == FT - 1),
                    )
                nc.vector.tensor_add(
                    res[:, n * NH:(n + 1) * NH], ps[:],
                    bias_t[:, n * NH:(n + 1) * NH])
            eng = nc.sync if i % 2 == 0 else nc.scalar
            eng.dma_start(out=out_d[i * P:(i + 1) * P, :], in_=res[:])

    nc.compile()
    return nc


class TileBuild:
    """TileContext + ExitStack pools in one with-block."""

    def __init__(self, nc):
        self.nc = nc
        self.ctx = ExitStack()
        self._tc_cm = tile.TileContext(nc, pool_alloc_mode="queue")

    def __enter__(self):
        self.tc = self._tc_cm.__enter__()
        self.ctx.__enter__()
        return self

    def __exit__(self, *exc):
        self.ctx.__exit__(*exc)
        return self._tc_cm.__exit__(*exc)


_NC_CACHE = None


def _get_nc():
    global _NC_CACHE
    if _NC_CACHE is None:
        _NC_CACHE = build_attention_nc()
    return _NC_CACHE


def prepare_inputs(x, y, w_qkv, w_out, b_out):
    bf16 = ml_dtypes.bfloat16
    xT = np.ascontiguousarray(np.transpose(x, (0, 2, 1))).astype(bf16)
    yT = np.ascontiguousarray(np.transpose(y, (0, 2, 1))).astype(bf16)
    wq = np.array(w_qkv, dtype=np.float32, copy=True)
    wq[0:DIM, :] *= SCALE  # fold softmax scale into the Q projection
    wqkvT = np.ascontiguousarray(wq.T)
    # wqkR[t, p, c, which, col]: pair-t [Q_t | K_t] column blocks, laid out
    # so each pair is one contiguous full-bandwidth [128, 2048] DMA whose
    # SBUF tile is c-major ([c*256 + which*128 + col] per partition p).
    wqk5 = wqkvT[:, 0:2 * DIM].reshape(
        CT_HOST, P, 2, FT_HOST, P)  # [c, p, which, t, col]
    wqkP = np.ascontiguousarray(
        wqk5.transpose(3, 1, 0, 2, 4)  # -> [t, p, c, which, col]
        .reshape(FT_HOST, P, 2 * DIM)).astype(bf16)
    wvT = np.ascontiguousarray(wqkvT[:, 2 * DIM:3 * DIM]).astype(bf16)
    woutT = np.ascontiguousarray(np.array(w_out, dtype=np.float32).T).astype(bf16)
    biasb = np.ascontiguousarray(
        np.broadcast_to(np.array(b_out, dtype=np.float32), (P, DIM)))
    in_maps = []
    for i in range(x.shape[0]):
        in_maps.append({
            "xT": np.ascontiguousarray(xT[i]),
            "yT": np.ascontiguousarray(yT[i]),
            "wqkP": wqkP,
            "wvT": wvT,
            "woutT": woutT,
            "biasb": biasb,
        })
    return in_maps


FT_HOST = DIM // P
CT_HOST = DIM // P


def kernel(x, y, w_qkv, w_out, b_out, trace=False):
    nc = _get_nc()
    in_maps = prepare_inputs(x, y, w_qkv, w_out, b_out)
    r = run_bass_kernel_spmd(nc, in_maps, core_ids=list(range(len(in_maps))),
                             trace=trace)
    out = np.stack([r.results[i]["out"] for i in range(len(in_maps))])
    if trace:
        kernel.last_results = r
    return out.astype(np.float32)


# revision 88
# speedup vs baseline: 1.0035x; 1.0035x over previous
"""Trainium2 Bass kernel: batched multi-head cross-attention.

Reference computation (per batch element b of 8, one NeuronCore each):
    K,V from x; Q from y (heads=16, dim=1024, d_head=64, scale=dim**-0.5)
    out = softmax(Q K^T * scale) V  -> concat heads -> @ w_out.T + b_out

Sharding: pure data-parallel on batch (8 batch elements -> 8 cores).
No collectives. All layout transposes are done host-side:

  xT, yT   : [dim, n]        (feature-major inputs)
  wqkP     : [dim, 16*128]   w_qkv.T Q|K columns regrouped per head pair:
                             block 2t = Q cols of pair t (pre-scaled),
                             block 2t+1 = K cols of pair t.  Pair-t blocks
                             are DMA'd t-ordered so pair 0/1 projections
                             (which gate the whole exp stream) start first.
  wvT      : [dim, dim]
  woutT    : [dim, dim]      = w_out.T   (loaded late, reuses wv space)
  biasb    : [128, dim]      = b_out broadcast over partitions

Device pipeline per core (bf16 matmuls, fp32 PSUM accumulation):
  Q^T[f,i] = wqT.T @ yT            (scale folded into wqT on host)
  K^T[f,j] = wkT.T @ xT
  V[j,f]   = xT.T @ wvT            (token-major, + ones column per head)
  dots^T[j,i] = k_h^T.T @ q_h^T    (K=64) -- the even/odd heads of a pair
      write the two halves of ONE [128,1024] psum tile, so both matmuls
      share a single dependency gate, issue back-to-back, and execute
      CONCURRENTLY on disjoint PE row strips (0,0)/(64,0): 2x dots rate.
  exp on ScalarE psum->sbuf bf16   ([128,1024] activations; ScalarE is the
      co-critical engine at ~144us total)
  [out_h^T; s_h] = [v_h|1].T @ exp   (M=65: row 64 = softmax denominator)
  out_h^T *= 1/s_h   (reciprocal_approx_fast + gpsimd partition_broadcast)
  res[i,g] = O^T.T @ woutT + bias    (natural layout, contiguous DMA out)

Scheduling: engine queues are in-order, so PE emission order ~= PE
execution order.  The dots+exp stream is ScalarE-paced, so between dots
windows we pump "filler" PE units (attn@V of the previous pair first --
that recycles ex tiles -- then V projection / Q,K projection two pairs
ahead) from a deque.
"""

from collections import deque
from contextlib import ExitStack

import numpy as np
import ml_dtypes

import concourse.bass as bass
import concourse.tile as tile
from concourse import bacc, mybir
from concourse.bass_utils import run_bass_kernel_spmd

DIM = 1024
N = 1024
HEADS = 16
DH = 64  # head dim
SCALE = DIM ** -0.5
P = 128          # partitions
NH = 512         # free-dim half (one PSUM bank of fp32)
BF16 = mybir.dt.bfloat16
F32 = mybir.dt.float32
EXP = mybir.ActivationFunctionType.Exp


def build_attention_nc():
    nc = bacc.Bacc("TRN2", target_bir_lowering=False, debug=False)

    xT_d = nc.dram_tensor("xT", [DIM, N], BF16, kind="ExternalInput")
    yT_d = nc.dram_tensor("yT", [DIM, N], BF16, kind="ExternalInput")
    # wqkR[t, p, c, col]: pair-t block, partition p, contraction tile c.
    # One full-bandwidth [128, 2048] DMA per head pair.
    wqkP_d = nc.dram_tensor("wqkP", [FT_HOST, P, 2 * DIM], BF16,
                            kind="ExternalInput")
    wvT_d = nc.dram_tensor("wvT", [DIM, DIM], BF16, kind="ExternalInput")
    woutT_d = nc.dram_tensor("woutT", [DIM, DIM], BF16, kind="ExternalInput")
    biasb_d = nc.dram_tensor("biasb", [P, DIM], F32, kind="ExternalInput")
    out_d = nc.dram_tensor("out", [N, DIM], F32, kind="ExternalOutput")

    CT = DIM // P   # 8 contraction tiles
    FT = DIM // P   # 8 head pairs
    JT = N // P     # 8 key-token tiles
    IT = N // P     # 8 query-token tiles

    with TileBuild(nc) as b:
        tc = b.tc
        ctx = b.ctx

        wqk_pool = ctx.enter_context(tc.tile_pool(name="wqk", bufs=FT))
        xy_pool = ctx.enter_context(tc.tile_pool(name="xy", bufs=2 * CT))
        qk_pool = ctx.enter_context(tc.tile_pool(name="qk", bufs=6))
        va_pool = ctx.enter_context(tc.tile_pool(name="va", bufs=JT))
        ex_pool = ctx.enter_context(tc.tile_pool(name="ex", bufs=20))
        ot_pool = ctx.enter_context(tc.tile_pool(name="ot", bufs=FT))
        sm_pool = ctx.enter_context(tc.tile_pool(name="sm", bufs=2))
        res_pool = ctx.enter_context(tc.tile_pool(name="res", bufs=2))
        # PSUM: 8 banks of [128, 2KB].  pd: dots pair tiles ([128,1024] f32
        # = 2 banks, 2 in flight), pp: projections/outproj ([128,512], 2),
        # pa: attn@V accumulators ([128,512], 2).
        pd_pool = ctx.enter_context(tc.tile_pool(name="pd", bufs=2, space="PSUM"))
        pp_pool = ctx.enter_context(tc.tile_pool(name="pp", bufs=2, space="PSUM"))
        pa_pool = ctx.enter_context(tc.tile_pool(name="pa", bufs=2, space="PSUM"))

        # ---- input DMA, two queues ----
        # sync queue:   wqk pair 0, pair 1, y, wqk pairs 2-7, bias
        # scalar queue: x, wv, (later) wout
        # DMA-completion waits are per-queue prefix-coarse, so the K
        # projections of pairs 0/1 unblock after ~1MB+x, Q after +y.
        wv_pool = ctx.enter_context(tc.tile_pool(name="wv", bufs=CT))
        wo_pool = ctx.enter_context(tc.tile_pool(name="wo", bufs=FT))
        wv_t, xT_t, yT_t = [], [], []
        wqk_t = []  # [t] -> [128, 2048] (c-major blocks of 256: [Q|K] cols)

        def dma_wqk(t):
            w = wqk_pool.tile([P, 2 * DIM], BF16, tag="wqk", name=f"wqk{t}")
            nc.sync.dma_start(w[:], wqkP_d[t, :, :])
            wqk_t.append(w)

        dma_wqk(0)
        dma_wqk(1)
        for c in range(CT):
            t = xy_pool.tile([P, N], BF16, tag="xy", name=f"xt{c}")
            nc.scalar.dma_start(t[:], xT_d[c * P:(c + 1) * P, :])
            xT_t.append(t)
        for c in range(CT):
            t = xy_pool.tile([P, N], BF16, tag="xy", name=f"yt{c}")
            nc.scalar.dma_start(t[:], yT_d[c * P:(c + 1) * P, :])
            yT_t.append(t)
        for c in range(CT):
            t = wv_pool.tile([P, DIM], BF16, tag="wv", name=f"wv{c}")
            nc.scalar.dma_start(t[:], wvT_d[c * P:(c + 1) * P, :])
            wv_t.append(t)
        for t in range(2, FT):
            dma_wqk(t)
        wo_t = []
        for f in range(FT):
            w = wo_pool.tile([P, DIM], BF16, tag="wo", name=f"wo{f}")
            nc.sync.dma_start(w[:], woutT_d[f * P:(f + 1) * P, :])
            wo_t.append(w)
        bias_t = res_pool.tile([P, DIM], F32, tag="bias", bufs=1)
        nc.sync.dma_start(bias_t[:], biasb_d[:, :])

        # ---- filler units: callables emitting ~8 PE matmuls each ----
        # Two priorities: attn@V units (hi) unblock the ex-tile ring, so
        # they preempt the projection backlog (lo) -- but only after aging
        # two dots windows, so the exp tiles they read have drained and
        # their matmuls don't stall the in-order PE queue.
        filler = deque()      # lo: projections / V projection
        filler_hi = deque()   # hi: (enqueue_window, attn@V unit)
        win = [0]

        def pump(k):
            for _ in range(k):
                if filler_hi and win[0] - filler_hi[0][0] >= 2:
                    filler_hi.popleft()[1]()
                elif filler:
                    filler.popleft()()
                elif filler_hi:
                    filler_hi.popleft()[1]()

        # Q/K projection for head pair t, one (which, half) per unit.
        QK = {}  # t -> [QTt, KTt]
        proj_pending = {}  # t -> number of un-run units

        def enqueue_proj(t):
            proj_pending[t] = 4
            QK[t] = [
                qk_pool.tile([P, N], BF16, tag="qk", name=f"qk{w}_{t}")
                for w in range(2)
            ]

            qk_tiles = QK[t]

            def unit(which, n, t=t):
                rhs_t = yT_t if which == 0 else xT_t
                ps = pp_pool.tile([P, NH], F32, tag="pp", name=f"psp{which}{n}_{t}")
                for c in range(CT):
                    nc.tensor.matmul(
                        ps[:],
                        lhsT=wqk_t[t][:, c * 256 + which * P:
                                       c * 256 + (which + 1) * P],
                        rhs=rhs_t[c][:, n * NH:(n + 1) * NH],
                        start=(c == 0), stop=(c == CT - 1),
                    )
                nc.vector.tensor_copy(
                    qk_tiles[which][:, n * NH:(n + 1) * NH], ps[:])
                proj_pending[t] -= 1

            # K (from x) first: the x DMA queue finishes before y's.
            for which in (1, 0):
                for n in range(2):
                    filler.append(lambda w=which, n=n: unit(w, n))

        # V projection for key tile vj: token-major + per-head ones column.
        VA = [None] * JT
        vproj_pending = [0]

        def enqueue_vproj(vj):
            vproj_pending[0] += 2
            va = va_pool.tile([P, HEADS, DH + 1], BF16, tag="va", name=f"va{vj}")
            VA[vj] = va

            def unit(n, vj=vj, va=va):
                ps = pp_pool.tile([P, NH], F32, tag="pp", name=f"psv{vj}_{n}")
                for c in range(CT):
                    nc.tensor.matmul(
                        ps[:],
                        lhsT=xT_t[c][:, vj * P:(vj + 1) * P],
                        rhs=wv_t[c][:, n * NH:(n + 1) * NH],
                        start=(c == 0), stop=(c == CT - 1),
                    )
                # half n covers heads 8n..8n+7 (512 = 8 heads x 64)
                nc.vector.tensor_copy(
                    va[:, 8 * n:8 * (n + 1), 0:DH],
                    ps[:].rearrange("p (h c) -> p h c", c=DH),
                )
                if n == 1:
                    nc.vector.memset(va[:, :, DH:DH + 1], 1.0)
                vproj_pending[0] -= 1

            for n in range(2):
                filler.append(lambda n=n: unit(n))

        # attn @ V + softmax-normalize for one head pair.  ex_t[n][j] is the
        # shared exp tile of (key tile j, half n); head par reads columns
        # [par*512:(par+1)*512].  The ones column of VA makes psum row 64
        # the softmax denominator.
        OT = [ot_pool.tile([P, N], BF16, tag="ot", name=f"OT{f}") for f in range(FT)]

        def enqueue_attnv(t, ex_t):
            def unit(par, n, t=t):
                h = 2 * t + par
                acc = pa_pool.tile([P, NH], F32, tag="pa", name=f"acc{h}_{n}")
                for j in range(JT):
                    nc.tensor.matmul(
                        acc[0:DH + 1, :],
                        lhsT=VA[j][:, h, :],
                        rhs=ex_t[n][j][:, par * NH:(par + 1) * NH],
                        start=(j == 0), stop=(j == JT - 1),
                    )
                # r = 1/s; HW custom ops only honor partition base 0, so
                # hop s: psum[64] -> sbuf[64] -> DMA -> sbuf[0].
                s_sb = sm_pool.tile([DH + 1, NH], F32, tag="ss", name=f"ss{h}_{n}")
                nc.vector.tensor_copy(s_sb[DH:DH + 1, :], acc[DH:DH + 1, :])
                r_sb = sm_pool.tile([1, NH], F32, tag="rs", name=f"rs{h}_{n}")
                nc.sync.dma_start(out=r_sb[0:1, :], in_=s_sb[DH:DH + 1, :])
                # recip result parked in s_sb's unused partition-0 row
                nc.vector.reciprocal_approx_fast(
                    out=s_sb[0:1, 0:NH], in_=r_sb[0:1, :])
                rb = sm_pool.tile([DH, NH], F32, tag="rb", name=f"rb{h}_{n}")
                nc.gpsimd.partition_broadcast(rb[:], s_sb[0:1, 0:NH])
                if par == 0:
                    nc.vector.tensor_mul(
                        OT[t][0:DH, n * NH:(n + 1) * NH], acc[0:DH, :], rb[:])
                else:
                    om = sm_pool.tile([DH, NH], BF16, tag="om", name=f"om{h}_{n}")
                    nc.vector.tensor_mul(om[:], acc[0:DH, :], rb[:])
                    nc.sync.dma_start(
                        out=OT[t][DH:P, n * NH:(n + 1) * NH], in_=om[:])

            return unit

        # ---- dots + exp for head pair t (the ScalarE-paced main stream).
        # One [128,1024] psum tile per (j, n): even head in cols 0:512, odd
        # head in cols 512:1024.  Both matmuls share the tile's dependency
        # gate -> issued back-to-back -> concurrent on row strips 0-1/2-3.
        def emit_dots(t):
            while proj_pending[t] > 0:
                pump(1)
            QTt, KTt = QK.pop(t)
            ex_t = ([], [])  # [n][j]
            attnv_unit = enqueue_attnv(t, ex_t)
            for n in range(2):
                for j in range(JT):
                    ps = pd_pool.tile([P, N], F32, tag="pd",
                                      name=f"psd{t}_{j}_{n}")
                    for par, pb in ((0, 0), (1, DH)):
                        nc.tensor.matmul(
                            ps[:, par * NH:(par + 1) * NH],
                            lhsT=KTt[pb:pb + DH, j * P:(j + 1) * P],
                            rhs=QTt[pb:pb + DH, n * NH:(n + 1) * NH],
                            start=True, stop=True,
                        )
                    ex = ex_pool.tile([P, N], BF16, tag="ex",
                                      name=f"ex{t}_{j}_{n}")
                    nc.scalar.activation(ex[:], ps[:], EXP)
                    ex_t[n].append(ex)
                    # this pair's half-n attn@V is enqueued once its 8 exp
                    # tiles are in flight (behind any still-pending V
                    # projections, which write the VA tiles it reads)
                    if j == JT - 1:
                        if vproj_pending[0] > 0:
                            filler.append(lambda nn=n: attnv_unit(0, nn))
                            filler.append(lambda nn=n: attnv_unit(1, nn))
                        else:
                            filler_hi.append((win[0], lambda nn=n: attnv_unit(0, nn)))
                            filler_hi.append((win[0], lambda nn=n: attnv_unit(1, nn)))
                    # one filler unit per window, but keep the first window
                    # of each pair clear so its dots/exp restart promptly
                    win[0] += 1
                    if 8 * n + j >= 1:
                        pump(1)
            return ex_t

        # ---- pipeline ----
        enqueue_proj(0)
        enqueue_proj(1)
        pump(4)  # pair-0 projection gates the stream; pair 1 fills early
        for t in range(FT):
            if t + 2 < FT:
                enqueue_proj(t + 2)
            if t == 0:
                for vj in range(JT):
                    enqueue_vproj(vj)
            emit_dots(t)
        pump(len(filler) + len(filler_hi))

        # ---- output projection + bias ----
        for i in range(IT):
            res = res_pool.tile([P, DIM], F32, tag="res", name=f"res{i}")
            for n in range(2):
                ps = pp_pool.tile([P, NH], F32, tag="pp", name=f"psf{i}_{n}")
                for f in range(FT):
                    nc.tensor.matmul(
                        ps[:],
                        lhsT=OT[f][:, i * P:(i + 1) * P],
                        rhs=wo_t[f][:, n * NH:(n + 1) * NH],
                        start=(f == 0), stop=(f == FT - 1),
                    )
                nc.vector.tensor_add(
                    res[:, n * NH:(n + 1) * NH], ps[:],
                    bias_t[:, n * NH:(n + 1) * NH])
            eng = nc.sync if i % 2 == 0 else nc.scalar
            eng.dma_start(out=out_d[i * P:(i + 1) * P, :], in_=res[:])

    nc.compile()
    return nc


class TileBuild:
    """TileContext + ExitStack pools in one with-block."""

    def __init__(self, nc):
        self.nc = nc
        self.ctx = ExitStack()
        self._tc_cm = tile.TileContext(nc)

    def __enter__(self):
        self.tc = self._tc_cm.__enter__()
        self.ctx.__enter__()
        return self

    def __exit__(self, *exc):
        self.ctx.__exit__(*exc)
        return self._tc_cm.__exit__(*exc)


_NC_CACHE = None


def _get_nc():
    global _NC_CACHE
    if _NC_CACHE is None:
        _NC_CACHE = build_attention_nc()
    return _NC_CACHE


def prepare_inputs(x, y, w_qkv, w_out, b_out):
    bf16 = ml_dtypes.bfloat16
    xT = np.ascontiguousarray(np.transpose(x, (0, 2, 1))).astype(bf16)
    yT = np.ascontiguousarray(np.transpose(y, (0, 2, 1))).astype(bf16)
    wq = np.array(w_qkv, dtype=np.float32, copy=True)
    wq[0:DIM, :] *= SCALE  # fold softmax scale into the Q projection
    wqkvT = np.ascontiguousarray(wq.T)
    # wqkR[t, p, c, which, col]: pair-t [Q_t | K_t] column blocks, laid out
    # so each pair is one contiguous full-bandwidth [128, 2048] DMA whose
    # SBUF tile is c-major ([c*256 + which*128 + col] per partition p).
    wqk5 = wqkvT[:, 0:2 * DIM].reshape(
        CT_HOST, P, 2, FT_HOST, P)  # [c, p, which, t, col]
    wqkP = np.ascontiguousarray(
        wqk5.transpose(3, 1, 0, 2, 4)  # -> [t, p, c, which, col]
        .reshape(FT_HOST, P, 2 * DIM)).astype(bf16)
    wvT = np.ascontiguousarray(wqkvT[:, 2 * DIM:3 * DIM]).astype(bf16)
    woutT = np.ascontiguousarray(np.array(w_out, dtype=np.float32).T).astype(bf16)
    biasb = np.ascontiguousarray(
        np.broadcast_to(np.array(b_out, dtype=np.float32), (P, DIM)))
    in_maps = []
    for i in range(x.shape[0]):
        in_maps.append({
            "xT": np.ascontiguousarray(xT[i]),
            "yT": np.ascontiguousarray(yT[i]),
            "wqkP": wqkP,
            "wvT": wvT,
            "woutT": woutT,
            "biasb": biasb,
        })
    return in_maps


FT_HOST = DIM // P
CT_HOST = DIM // P


def kernel(x, y, w_qkv, w_out, b_out, trace=False):
    nc = _get_nc()
    in_maps = prepare_inputs(x, y, w_qkv, w_out, b_out)
    r = run_bass_kernel_spmd(nc, in_maps, core_ids=list(range(len(in_maps))),
                             trace=trace)
    out = np.stack([r.results[i]["out"] for i in range(len(in_maps))])
    if trace:
        kernel.last_results = r
    return out.astype(np.float32)
